# revision 9
# baseline (speedup 1.0000x reference)
"""Trainium2 Bass kernel for nn_PointDecoderSimple.

Strategy: pure data parallel over batch (8 cores, 1 batch element each).
Host does the index math (pack -> labels/positions), embedding gathers and a
label-sort permutation of the 1024 tokens; each core runs a specialized
straight-line Bass/Tile program:

  - activations kept feature-major ([D, tok]) so every matmul contracts over
    partitions with zero transposes
  - ragged (label-blocked) self-attention: scores/exp/AV only computed on each
    key tile's segment window; per-run row masking via per-partition exp bias
  - dense cross-attention to the 1024 memory tokens
  - softmax denominators via col-packed ones-matmuls; scores via 4-head
    row-packed matmuls (one PSUM bank per head)
  - LN statistics via ones-matmul partition reductions; rstd as exp(-0.5*ln v)
    so the scalar engine only ever uses the natural_log/exp table set

Returns (out [8,1024,3] float32, labels [8,1024] int32) matching the
reference.
"""

import numpy as np
import ml_dtypes

# ---------------------------------------------------------------------------
# constants
# ---------------------------------------------------------------------------
B, N, D, H, DH, NL, MEM, FF, PART = 8, 1024, 256, 8, 32, 4, 1024, 1024, 4
NT = N // 128           # token tiles
DT = D // 128           # feature tiles
QB = 512                # q block for attention passes
SCALE = float(1.0 / np.sqrt(DH))
NEGB = -30000.0
EPS = 1e-5
F32 = None  # filled after concourse import
BF16 = None

_jax_ready = False


def _setup_jax():
    global _jax_ready
    if _jax_ready:
        return
    import jax
    try:
        jax.config.update("jax_compilation_cache_dir", "/root/jax_cache")
        jax.config.update("jax_persistent_cache_min_compile_time_secs", 0.0)
        jax.config.update("jax_persistent_cache_min_entry_size_bytes", 0)
    except Exception:
        pass
    _jax_ready = True


# ---------------------------------------------------------------------------
# Tile tail-drain + per-instruction wait-count workarounds for this walrus
# ---------------------------------------------------------------------------
_patched = False


def _apply_patches():
    global _patched
    if _patched:
        return
    import bass_rust
    from concourse import tile as _tile
    from concourse.vector_clock import ScopedClock

    def _chunked_drain_and_barrier(self, tick_clock, wait_clock):
        nc = self.nc
        gc = tick_clock.global_clock
        ticks = [gc.peek_next(p) - 1 for p in range(27)]
        live = [p for p in range(27) if ticks[p] > 0]
        for p in live:
            vc = bass_rust.VectorClock()
            vc.require_at_least(p, ticks[p])
            d = nc.sync.nop(hint="tail_wait", nofuse=True)
            wait_clock.add_sem_waits(d.ins, ScopedClock({None: vc}))
        nc.sync.drain()
        nc.all_engine_barrier()
        assert self.sems is not None
        popped = nc._tile_sem_poison_stack.pop()
        assert popped is self._sem_poison
        nc.clear_and_free_semaphores(list(self.sems.allocated().values()))
        nc.all_engine_barrier()

    _tile.TileContext._drain_and_barrier = _chunked_drain_and_barrier
    _patched = True


_CTRL_OPCODES = {"NoOp", "Drain", "EventSemaphore"}


def _split_excess_waits(nc, max_compute=1, max_ctrl=1):
    import concourse.mybir as mybir

    engs = {
        mybir.EngineType.PE: nc.tensor,
        mybir.EngineType.Activation: nc.scalar,
        mybir.EngineType.DVE: nc.vector,
        mybir.EngineType.SP: nc.sync,
        mybir.EngineType.Pool: nc.gpsimd,
    }

    def make_nop(engine, waits):
        bi = engs[engine].nop(hint="waitfix", nofuse=True)
        nop = bi.ins
        bb = nc.cur_bb.bb
        assert bb.instructions and bb.instructions[-1] is nop
        bb.instructions.pop()
        nop.sync_info = mybir.SyncInfo(on_wait=list(waits), on_update=[])
        return nop

    n_fixed = 0
    for f in nc.m.functions:
        for blk in f.blocks:
            new = []
            for inst in blk.instructions:
                si = inst.sync_info
                limit = max_ctrl if inst.opcode in _CTRL_OPCODES else max_compute
                if si is not None and si.on_wait and len(si.on_wait) > limit:
                    waits = list(si.on_wait)
                    keep, excess = waits[:limit], waits[limit:]
                    for j in range(0, len(excess), max_ctrl):
                        new.append(make_nop(inst.engine, excess[j:j + max_ctrl]))
                    inst.sync_info = mybir.SyncInfo(
                        on_wait=keep, on_update=list(si.on_update or []))
                    n_fixed += 1
                new.append(inst)
            blk.instructions[:] = new
    return n_fixed


# ---------------------------------------------------------------------------
# host-side pack (numpy port of reference _pack)
# ---------------------------------------------------------------------------
def _pack_np(point_ratio):
    pr = np.asarray(point_ratio, np.float32)
    bsz, p = pr.shape
    pp = np.ceil(pr * N).astype(np.int64)
    cum = np.minimum(np.cumsum(pp, axis=1), N).astype(np.int64)
    prev = np.concatenate([np.zeros((bsz, 1), np.int64), cum[:, :-1]], axis=1)
    t = np.arange(N)
    raw = (t[None, None, :] >= cum[:, :, None]).sum(1)
    valid = raw < p
    labels = np.where(valid, raw, 0)
    start = np.take_along_axis(prev, labels, axis=1)
    position = np.where(valid, t[None, :] - start, 0)
    return labels.astype(np.int32), position.astype(np.int32)


# ---------------------------------------------------------------------------
# per-core segment metadata
# ---------------------------------------------------------------------------
def _segment_meta(sorted_labels):
    counts = np.bincount(sorted_labels, minlength=PART)
    bnd = np.concatenate([[0], np.cumsum(counts)]).astype(int)  # 5 entries
    runs_by_kt = []
    for kt in range(NT):
        t0, t1 = kt * 128, (kt + 1) * 128
        runs = []
        for s in range(PART):
            lo, hi = max(bnd[s], t0), min(bnd[s + 1], t1)
            if lo < hi:
                runs.append((lo - t0, hi - t0, int(bnd[s]), int(bnd[s + 1])))
        runs_by_kt.append(runs)
    return tuple(bnd), tuple(tuple(r) for r in runs_by_kt)


# ---------------------------------------------------------------------------
# the per-core program
# ---------------------------------------------------------------------------
_prog_cache = {}


def _build_program(runs_by_kt, n_sbias):
    import concourse.bass as bass
    import concourse.mybir as mybir
    import concourse.tile as tile
    from contextlib import ExitStack

    _apply_patches()
    F32, BF16 = mybir.dt.float32, mybir.dt.bfloat16
    EXP = mybir.ActivationFunctionType.Exp
    LN_ = mybir.ActivationFunctionType.Ln
    ADD = mybir.AluOpType.add
    MULT = mybir.AluOpType.mult
    MAX = mybir.AluOpType.max

    nc = bass.Bass()
    # ---- dram I/O ----
    XF = nc.dram_tensor("XF", [D, N], F32, kind="ExternalInput")
    MEMT = nc.dram_tensor("MEMT", [D, MEM], BF16, kind="ExternalInput")
    CBIAS = nc.dram_tensor("CBIAS", [128, NT], F32, kind="ExternalInput")
    SBIAS = nc.dram_tensor("SBIAS", [128, max(1, n_sbias)], F32, kind="ExternalInput")
    wnames = []
    for l in range(NL):
        for w in ("wqs", "wks", "wvs", "wos", "wqc", "wkc", "wvc", "woc"):
            wnames.append((f"L{l}_{w}", [128, 2 * 256], BF16))
        wnames.append((f"L{l}_w1", [128, 2 * FF], BF16))
        wnames.append((f"L{l}_w2", [128, (FF // 128) * 256], BF16))
        wnames.append((f"L{l}_b1", [128, FF // 128], F32))
        wnames.append((f"L{l}_b2", [128, 2], F32))
    wnames += [("wf1", [128, 2 * 128], BF16), ("wf2", [128, 32], BF16),
               ("wf3", [32, 3], BF16), ("bf1", [128, 1], F32),
               ("bf2", [32, 1], F32), ("bf3", [3, 1], F32)]
    dram_w = {nm: nc.dram_tensor(nm, sh, dt, kind="ExternalInput")
              for nm, sh, dt in wnames}
    OUTF = nc.dram_tensor("OUTF", [3, N], F32, kind="ExternalOutput")

    Wkt = []
    for kt in range(NT):
        runs = runs_by_kt[kt]
        Wkt.append((min(r[2] for r in runs), max(r[3] for r in runs)))

    with tile.TileContext(nc) as tc, ExitStack() as ctx:
        wts = ctx.enter_context(tc.tile_pool(name="wts", bufs=1))
        resid = ctx.enter_context(tc.tile_pool(name="resid", bufs=1))
        act = ctx.enter_context(tc.tile_pool(name="act", bufs=1))
        small = ctx.enter_context(tc.tile_pool(name="small", bufs=1))
        espool = ctx.enter_context(tc.tile_pool(name="espool", bufs=2))
        hpool = ctx.enter_context(tc.tile_pool(name="hpool", bufs=1))
        big = ctx.enter_context(tc.tile_pool(name="bigp", bufs=1, space="PSUM"))
        acc = ctx.enter_context(tc.tile_pool(name="accp", bufs=4, space="PSUM"))

        # ---- load weights & data ----
        wsb = {}
        for nm, sh, dt in wnames:
            t = wts.tile(sh, dt, tag=nm, name=nm)
            nc.sync.dma_start(out=t, in_=dram_w[nm][:, :])
            wsb[nm] = t
        x = [resid.tile([128, N], F32, tag=f"x{dti}", name=f"x{dti}") for dti in range(DT)]
        for dti in range(DT):
            nc.sync.dma_start(out=x[dti], in_=XF[dti * 128:(dti + 1) * 128, :])
        memt = [wts.tile([128, MEM], BF16, tag=f"memt{dti}", name=f"memt{dti}") for dti in range(DT)]
        for dti in range(DT):
            nc.sync.dma_start(out=memt[dti], in_=MEMT[dti * 128:(dti + 1) * 128, :])
        cbias = wts.tile([128, NT], F32, tag="cbias", name="cbias")
        nc.sync.dma_start(out=cbias, in_=CBIAS[:, :])
        sbias = wts.tile([128, max(1, n_sbias)], F32, tag="sbias", name="sbias")
        nc.sync.dma_start(out=sbias, in_=SBIAS[:, :])

        # ---- constants ----
        onesA = wts.tile([128, 33], BF16, tag="onesA", name="onesA")   # col0 = 1
        nc.vector.memset(onesA, 0.0)
        nc.vector.memset(onesA[:, 0:1], 1.0)
        onesB = wts.tile([128, 33], BF16, tag="onesB", name="onesB")   # col32 = 1
        nc.vector.memset(onesB, 0.0)
        nc.vector.memset(onesB[:, 32:33], 1.0)
        ones1 = wts.tile([128, 1], BF16, tag="ones1", name="ones1")
        nc.vector.memset(ones1, 1.0)
        onesr = wts.tile([1, 128], BF16, tag="onesr", name="onesr")
        nc.vector.memset(onesr, 1.0)
        e4p = wts.tile([128, 128], BF16, tag="e4p", name="e4p")
        epsT = wts.tile([1, 1], F32, tag="epsT", name="epsT")
        nc.vector.memset(epsT, EPS)
        nc.vector.memset(e4p, 0.0)
        for h in range(4):
            nc.vector.memset(e4p[32 * h:32 * h + 1, 32 * h:32 * h + 32], 1.0)

        # -------------------------------------------------------------------
        def ln256(xt, nd, tagp):
            """LayerNorm over D=256 in feature-major layout -> bf16 tiles."""
            inv = 1.0 / nd
            xb = [act.tile([128, N], BF16, tag=f"{tagp}xb{i}", name=f"{tagp}xb{i}") for i in range(DT)]
            sq = [act.tile([128, N], BF16, tag=f"{tagp}sq{i}", name=f"{tagp}sq{i}") for i in range(DT)]
            for i in range(DT):
                nc.vector.tensor_copy(xb[i], xt[i])
                nc.vector.tensor_mul(sq[i], xb[i], xb[i])
            S = big.tile([33, N], F32, tag="big", name="big")
            for half in range(2):
                sl = slice(half * QB, (half + 1) * QB)
                nc.tensor.matmul(S[:, sl], onesA, xb[0][:, sl], start=True, stop=False)
                nc.tensor.matmul(S[:, sl], onesA, xb[1][:, sl], start=False, stop=False)
                nc.tensor.matmul(S[:, sl], onesB, sq[0][:, sl], start=False, stop=False)
                nc.tensor.matmul(S[:, sl], onesB, sq[1][:, sl], start=False, stop=True)
            mn = small.tile([1, N], F32, tag="ln_mn", name="ln_mn")
            vn = small.tile([1, N], F32, tag="ln_vn", name="ln_vn")
            nc.vector.tensor_scalar(mn, S[0:1, :], -inv, None, MULT)  # -mean
            nc.vector.tensor_scalar(vn, S[32:33, :], inv, None, MULT)  # E[x^2]
            msq = small.tile([1, N], F32, tag="ln_msq", name="ln_msq")
            nc.vector.tensor_mul(msq, mn, mn)
            var = small.tile([1, N], F32, tag="ln_var", name="ln_var")
            nc.vector.tensor_sub(var, vn, msq)
            lnv = small.tile([1, N], F32, tag="ln_lnv", name="ln_lnv")
            nc.scalar.activation(lnv, var, LN_, bias=epsT[0:1, 0:1], scale=1.0)
            aT = small.tile([1, N], BF16, tag="ln_aT", name="ln_aT")
            nc.scalar.activation(aT, lnv, EXP, bias=0.0, scale=-0.5)  # rstd
            bT = small.tile([1, N], BF16, tag="ln_bT", name="ln_bT")
            nc.vector.tensor_mul(bT, mn, aT)                          # -mean*rstd
            xn = [act.tile([128, N], BF16, tag=f"{tagp}xn{i}", name=f"{tagp}xn{i}") for i in range(DT)]
            for half in range(2):
                sl = slice(half * QB, (half + 1) * QB)
                aB = acc.tile([128, QB], F32, tag="acc", name="acc")
                bB = acc.tile([128, QB], F32, tag="acc", name="acc")
                nc.tensor.matmul(aB, onesr, aT[0:1, sl], start=True, stop=True)
                nc.tensor.matmul(bB, onesr, bT[0:1, sl], start=True, stop=True)
                ra = small.tile([128, QB], BF16, tag="ln_ra", name="ln_ra")
                rb = small.tile([128, QB], BF16, tag="ln_rb", name="ln_rb")
                nc.vector.tensor_copy(ra, aB)
                nc.vector.tensor_copy(rb, bB)
                for i in range(DT):
                    tmp = small.tile([128, QB], BF16, tag="ln_tmp", name="ln_tmp")
                    nc.vector.tensor_mul(tmp, xb[i][:, sl], ra)
                    nc.vector.tensor_add(xn[i][:, sl], tmp, rb)
            return xn

        def proj_fm(wname, src, tagp):
            """Feature-major projection: out[g][:,:] = (W^T @ src) bf16."""
            w = wsb[wname]
            out = [act.tile([128, N], BF16, tag=f"{tagp}{g}", name=f"{tagp}{g}") for g in range(2)]
            for g in range(2):
                for half in range(2):
                    sl = slice(half * QB, (half + 1) * QB)
                    ps = acc.tile([128, QB], F32, tag="acc", name="acc")
                    for dti in range(DT):
                        nc.tensor.matmul(
                            ps, w[:, dti * 256 + g * 128: dti * 256 + g * 128 + 128],
                            src[dti][:, sl], start=(dti == 0), stop=(dti == DT - 1))
                    nc.vector.tensor_copy(out[g][:, sl], ps)
            return out

        def proj_tm(wname, src, ntk, tagp):
            """Token-major projection (for V): out [128, ntk, 256] bf16."""
            w = wsb[wname]
            out = act.tile([128, ntk, 256], BF16, tag=tagp, name=tagp)
            for kt in range(ntk):
                ps = acc.tile([128, 256], F32, tag="acc", name="acc")
                for dti in range(DT):
                    nc.tensor.matmul(
                        ps, src[dti][:, kt * 128:(kt + 1) * 128],
                        w[:, dti * 256:(dti + 1) * 256],
                        start=(dti == 0), stop=(dti == DT - 1))
                nc.vector.tensor_copy(out[:, kt, :], ps)
            return out

        def attention(qT, kT, v, ktiles_for, es_bias, es_window, tagp):
            """Generic attention pass. Produces oT (2 tiles [128, N] bf16)."""
            oT = [act.tile([128, N], BF16, tag=f"{tagp}o{g}", name=f"{tagp}o{g}") for g in range(2)]
            for g in range(2):
                for qb in range(0, N, QB):
                    kts = ktiles_for(qb)
                    pav = acc.tile([128, QB], F32, tag="acc", name="acc")
                    pl = acc.tile([128, QB], F32, tag="acc", name="acc")
                    for idx, kt in enumerate(kts):
                        o0, o1 = es_window(kt, qb)
                        psc = big.tile([128, 4, 512], F32, tag="big", name="big")
                        for hh in range(4):
                            nc.tensor.matmul(
                                psc[:, hh, o0:o1],
                                kT[g][32 * hh:32 * hh + 32, kt * 128:(kt + 1) * 128],
                                qT[g][32 * hh:32 * hh + 32, qb + o0:qb + o1],
                                start=True, stop=True, tile_position=(32 * hh, 0))
                        es = espool.tile([128, 4, QB], BF16, tag="es", name="es")
                        es_bias(es, psc, kt, qb, o0, o1)
                        first, last = idx == 0, idx == len(kts) - 1
                        for hh in range(4):
                            nc.tensor.matmul(
                                pav[32 * hh:32 * hh + 32, :],
                                v[:, kt, 32 * (4 * g + hh):32 * (4 * g + hh) + 32],
                                es[:, hh, :], start=first, stop=last,
                                tile_position=(0, 32 * hh))
                            nc.tensor.matmul(
                                pl[32 * hh:32 * hh + 1, :], ones1, es[:, hh, :],
                                start=first, stop=last, tile_position=(0, 32 * hh))
                    lsb = small.tile([128, QB], BF16, tag="lsb", name="lsb")
                    nc.vector.tensor_copy(lsb, pl)
                    pb = acc.tile([128, QB], F32, tag="acc", name="acc")
                    nc.tensor.matmul(pb, e4p, lsb, start=True, stop=True)
                    rb = small.tile([128, QB], F32, tag="rbr", name="rbr")
                    nc.vector.reciprocal(rb, pb)
                    nc.vector.tensor_mul(oT[g][:, qb:qb + QB], pav, rb)
            return oT

        def wo_residual(wname, oT):
            w = wsb[wname]
            for ob in range(2):
                for half in range(2):
                    sl = slice(half * QB, (half + 1) * QB)
                    ps = acc.tile([128, QB], F32, tag="acc", name="acc")
                    for g in range(2):
                        nc.tensor.matmul(
                            ps, w[:, g * 256 + ob * 128: g * 256 + ob * 128 + 128],
                            oT[g][:, sl], start=(g == 0), stop=(g == 1))
                    nc.vector.tensor_add(x[ob][:, sl], x[ob][:, sl], ps)

        # ---- bias index bookkeeping for ragged self-attention ----
        bias_idx = {}
        nb = 0
        for kt in range(NT):
            runs = runs_by_kt[kt]
            if len(runs) == 1 and runs[0][0] == 0 and runs[0][1] == 128:
                continue
            for ri in range(len(runs)):
                bias_idx[(kt, ri)] = nb
                nb += 1
        assert nb == n_sbias, (nb, n_sbias)

        # ===================================================================
        for l in range(NL):
            # ---- self attention ----
            xn1 = ln256(x, D, "")
            qT = proj_fm(f"L{l}_wqs", xn1, "q")
            kT = proj_fm(f"L{l}_wks", xn1, "k")
            vS = proj_tm(f"L{l}_wvs", xn1, NT, "vtm")

            def self_kts(qb):
                return [kt for kt in range(NT)
                        if Wkt[kt][0] < qb + QB and Wkt[kt][1] > qb]

            def self_window(kt, qb):
                w0, w1 = Wkt[kt]
                return max(w0 - qb, 0), min(w1 - qb, QB)

            def self_bias(es, psc, kt, qb, o0, o1):
                runs = runs_by_kt[kt]
                if (o0, o1) != (0, QB) or len(runs) > 1:
                    nc.vector.memset(es, 0.0)
                for ri, (r0, r1, w0, w1) in enumerate(runs):
                    v0, v1 = max(w0 - qb, 0), min(w1 - qb, QB)
                    if v0 >= v1:
                        continue
                    if (kt, ri) in bias_idx:
                        bi = bias_idx[(kt, ri)]
                        bias = sbias[:, bi:bi + 1]
                    else:
                        bias = 0.0
                    nc.scalar.activation(es[:, :, v0:v1], psc[:, :, v0:v1],
                                         EXP, bias=bias, scale=SCALE)

            oT = attention(qT, kT, vS, self_kts, self_bias, self_window, "at")
            wo_residual(f"L{l}_wos", oT)

            # ---- cross attention ----
            xn2 = ln256(x, D, "")
            qcT = proj_fm(f"L{l}_wqc", xn2, "q")
            kcT = proj_fm(f"L{l}_wkc", memt, "k")
            vC = proj_tm(f"L{l}_wvc", memt, NT, "vtm")

            def cross_bias(es, psc, kt, qb, o0, o1):
                nc.scalar.activation(es[:, :, :], psc[:, :, 0:QB], EXP,
                                     bias=cbias[:, kt:kt + 1], scale=SCALE)

            oTc = attention(qcT, kcT, vC, lambda qb: list(range(NT)), cross_bias,
                            lambda kt, qb: (0, QB), "at")
            wo_residual(f"L{l}_woc", oTc)

            # ---- FFN ----
            xn3 = ln256(x, D, "")
            w1, w2 = wsb[f"L{l}_w1"], wsb[f"L{l}_w2"]
            b1, b2 = wsb[f"L{l}_b1"], wsb[f"L{l}_b2"]
            hb = [hpool.tile([128, N], BF16, tag=f"hb{f}", name=f"hb{f}") for f in range(FF // 128)]
            for f in range(FF // 128):
                for half in range(2):
                    sl = slice(half * QB, (half + 1) * QB)
                    ps = acc.tile([128, QB], F32, tag="acc", name="acc")
                    for dti in range(DT):
                        nc.tensor.matmul(
                            ps, w1[:, dti * FF + f * 128: dti * FF + f * 128 + 128],
                            xn3[dti][:, sl], start=(dti == 0), stop=(dti == DT - 1))
                    nc.vector.tensor_scalar(hb[f][:, sl], ps, b1[:, f:f + 1], 0.0,
                                            ADD, MAX)
            for ob in range(2):
                for half in range(2):
                    sl = slice(half * QB, (half + 1) * QB)
                    ps = acc.tile([128, QB], F32, tag="acc", name="acc")
                    for f in range(FF // 128):
                        nc.tensor.matmul(
                            ps, w2[:, f * 256 + ob * 128: f * 256 + ob * 128 + 128],
                            hb[f][:, sl], start=(f == 0), stop=(f == FF // 128 - 1))
                    t = small.tile([128, QB], F32, tag="ffn_t", name="ffn_t")
                    nc.vector.tensor_scalar(t, ps, b2[:, ob:ob + 1], None, ADD)
                    nc.vector.tensor_add(x[ob][:, sl], x[ob][:, sl], t)

        # ===================================================================
        # final block: Linear(256->128) -> LN -> ReLU -> Linear(128->32) -> LN
        # -> ReLU -> Linear(32->3)
        def final_ln_relu(raw_ps, nd, bias_t, tagp):
            """raw_ps: psum [nd_p, N]; returns bf16 [nd_p, N] = relu(LN(raw+b))."""
            nd_p = raw_ps.shape[0]
            hraw = act.tile([nd_p, N], BF16, tag=f"{tagp}hr", name=f"{tagp}hr")
            nc.vector.tensor_scalar(hraw, raw_ps, bias_t, None, ADD)
            sq = act.tile([nd_p, N], BF16, tag=f"{tagp}sq", name=f"{tagp}sq")
            nc.vector.tensor_mul(sq, hraw, hraw)
            S = big.tile([33, N], F32, tag="big", name="big")
            for half in range(2):
                sl = slice(half * QB, (half + 1) * QB)
                nc.tensor.matmul(S[:, sl], onesA[0:nd_p, :], hraw[:, sl],
                                 start=True, stop=False)
                nc.tensor.matmul(S[:, sl], onesB[0:nd_p, :], sq[:, sl],
                                 start=False, stop=True)
            inv = 1.0 / nd
            mn = small.tile([1, N], F32, tag="ln_mn", name="ln_mn")
            vn = small.tile([1, N], F32, tag="ln_vn", name="ln_vn")
            nc.vector.tensor_scalar(mn, S[0:1, :], -inv, None, MULT)
            nc.vector.tensor_scalar(vn, S[32:33, :], inv, None, MULT)
            msq = small.tile([1, N], F32, tag="ln_msq", name="ln_msq")
            nc.vector.tensor_mul(msq, mn, mn)
            var = small.tile([1, N], F32, tag="ln_var", name="ln_var")
            nc.vector.tensor_sub(var, vn, msq)
            lnv = small.tile([1, N], F32, tag="ln_lnv", name="ln_lnv")
            nc.scalar.activation(lnv, var, LN_, bias=epsT[0:1, 0:1], scale=1.0)
            aT = small.tile([1, N], BF16, tag="ln_aT", name="ln_aT")
            nc.scalar.activation(aT, lnv, EXP, bias=0.0, scale=-0.5)
            bT = small.tile([1, N], BF16, tag="ln_bT", name="ln_bT")
            nc.vector.tensor_mul(bT, mn, aT)
            out = act.tile([nd_p, N], BF16, tag=f"{tagp}out", name=f"{tagp}out")
            for half in range(2):
                sl = slice(half * QB, (half + 1) * QB)
                aB = acc.tile([128, QB], F32, tag="acc", name="acc")
                bB = acc.tile([128, QB], F32, tag="acc", name="acc")
                nc.tensor.matmul(aB[0:nd_p, :], onesr[:, 0:nd_p], aT[0:1, sl], start=True, stop=True)
                nc.tensor.matmul(bB[0:nd_p, :], onesr[:, 0:nd_p], bT[0:1, sl], start=True, stop=True)
                ra = small.tile([128, QB], BF16, tag="ln_ra", name="ln_ra")
                rb = small.tile([128, QB], BF16, tag="ln_rb", name="ln_rb")
                nc.vector.tensor_copy(ra[0:nd_p, :], aB[0:nd_p, :])
                nc.vector.tensor_copy(rb[0:nd_p, :], bB[0:nd_p, :])
                tmp = small.tile([128, QB], BF16, tag="ln_tmp", name="ln_tmp")
                nc.vector.tensor_mul(tmp[0:nd_p, :], hraw[:, sl], ra[0:nd_p, :])
                t2 = small.tile([128, QB], BF16, tag="ln_t2", name="ln_t2")
                nc.vector.tensor_add(t2[0:nd_p, :], tmp[0:nd_p, :], rb[0:nd_p, :])
                nc.vector.tensor_scalar(out[:, sl], t2[0:nd_p, :], 0.0, None, MAX)
            return out

        xb = [act.tile([128, N], BF16, tag=f"xb{i}", name=f"xb{i}") for i in range(DT)]
        for i in range(DT):
            nc.vector.tensor_copy(xb[i], x[i])
        wf1 = wsb["wf1"]
        p1 = big.tile([128, N], F32, tag="big", name="big")
        for half in range(2):
            sl = slice(half * QB, (half + 1) * QB)
            for dti in range(DT):
                nc.tensor.matmul(p1[:, sl], wf1[:, dti * 128:(dti + 1) * 128],
                                 xb[dti][:, sl], start=(dti == 0), stop=(dti == DT - 1))
        h1 = final_ln_relu(p1, 128, wsb["bf1"][:, 0:1], "f1")
        p2 = big.tile([32, N], F32, tag="big", name="big")
        for half in range(2):
            sl = slice(half * QB, (half + 1) * QB)
            nc.tensor.matmul(p2[:, sl], wsb["wf2"], h1[:, sl], start=True, stop=True)
        h2 = final_ln_relu(p2, 32, wsb["bf2"][:, 0:1], "f2")
        po = big.tile([3, N], F32, tag="big", name="big")
        for half in range(2):
            sl = slice(half * QB, (half + 1) * QB)
            nc.tensor.matmul(po[:, sl], wsb["wf3"], h2[:, sl], start=True, stop=True)
        osb = small.tile([3, N], F32, tag="osb", name="osb")
        nc.vector.tensor_scalar(osb, po, wsb["bf3"][:, 0:1], None, ADD)
        nc.sync.dma_start(out=OUTF[:, :], in_=osb)

    _split_excess_waits(nc)
    return nc


# ---------------------------------------------------------------------------
# host wrapper
# ---------------------------------------------------------------------------
def _prep_weight(params):
    """Shared (replicated) weight blobs, host-side layouts."""
    bf = ml_dtypes.bfloat16

    def fm(w):  # [Din, Dout] -> [128, (Din/128)*Dout]
        din, dout = w.shape
        return np.ascontiguousarray(
            w.reshape(din // 128, 128, dout).transpose(1, 0, 2).reshape(128, -1)
        ).astype(bf)

    out = {}
    for l in range(NL):
        for nm, key in (("wqs", "Wq_s"), ("wks", "Wk_s"), ("wvs", "Wv_s"),
                        ("wos", "Wo_s"), ("wqc", "Wq_c"), ("wkc", "Wk_c"),
                        ("wvc", "Wv_c"), ("woc", "Wo_c")):
            out[f"L{l}_{nm}"] = fm(np.asarray(params[key][l], np.float32))
        out[f"L{l}_w1"] = fm(np.asarray(params["W1"][l], np.float32))
        out[f"L{l}_w2"] = fm(np.asarray(params["W2"][l], np.float32))
        out[f"L{l}_b1"] = np.ascontiguousarray(
            np.asarray(params["b1"][l], np.float32).reshape(FF // 128, 128).T)
        out[f"L{l}_b2"] = np.ascontiguousarray(
            np.asarray(params["b2"][l], np.float32).reshape(2, 128).T)
    out["wf1"] = fm(np.asarray(params["Wf1"], np.float32))
    out["wf2"] = np.asarray(params["Wf2"], np.float32).astype(bf)
    out["wf3"] = np.asarray(params["Wf3"], np.float32).astype(bf)
    out["bf1"] = np.asarray(params["bf1"], np.float32).reshape(128, 1)
    out["bf2"] = np.asarray(params["bf2"], np.float32).reshape(32, 1)
    out["bf3"] = np.asarray(params["bf3"], np.float32).reshape(3, 1)
    return out


def kernel(encoder_output, input_point, input_label, input_ratio, point_ratio,
           n, encoder_output_full, mask, params):
    _setup_jax()
    import jax
    from concourse.bass_utils import run_bass_kernel_spmd

    bf = ml_dtypes.bfloat16
    enc = np.asarray(encoder_output, np.float32)
    encf = np.asarray(encoder_output_full, np.float32)
    msk = np.asarray(mask)
    pr = np.asarray(point_ratio, np.float32)
    pos_table = np.asarray(params["pos_table"], np.float32)
    label_table = np.asarray(params["label_table"], np.float32)

    labels, position = _pack_np(pr)
    wblobs = _prep_weight(params)

    core_data = []
    for b in range(B):
        lab = labels[b]
        perm = np.argsort(lab, kind="stable")
        x0 = pos_table[position[b]] + enc[b][lab] + label_table[lab]  # [N, D]
        xf = np.ascontiguousarray(x0[perm].T).astype(np.float32)     # [D, N]
        memt = np.ascontiguousarray(encf[b, PART:, :].T).astype(bf)  # [D, MEM]
        cb = np.where(msk[b, 0, PART:], 0.0, NEGB).astype(np.float32)
        cbias = np.ascontiguousarray(cb.reshape(NT, 128).T)          # [128, NT]
        bnd, runs_by_kt = _segment_meta(lab[perm])
        # self-attention run bias columns
        cols = []
        for kt in range(NT):
            runs = runs_by_kt[kt]
            if len(runs) == 1 and runs[0][0] == 0 and runs[0][1] == 128:
                continue
            for (r0, r1, w0, w1) in runs:
                col = np.full(128, NEGB, np.float32)
                col[r0:r1] = 0.0
                cols.append(col)
        sb = (np.stack(cols, 1) if cols else np.zeros((128, 1), np.float32))
        in_map = {"XF": xf, "MEMT": memt, "CBIAS": cbias,
                  "SBIAS": np.ascontiguousarray(sb)}
        in_map.update(wblobs)
        core_data.append((runs_by_kt, len(cols), in_map, perm))

    outs = [None] * B
    devices = jax.devices()
    for b in range(B):
        runs_by_kt, nsb, _, _ = core_data[b]
        key = (runs_by_kt, nsb)
        if key not in _prog_cache:
            _prog_cache[key] = _build_program(runs_by_kt, nsb)

    def run_core(b):
        runs_by_kt, nsb, in_map, perm = core_data[b]
        nc = _prog_cache[(runs_by_kt, nsb)]
        with jax.default_device(devices[b % len(devices)]):
            res = run_bass_kernel_spmd(nc, [in_map], core_ids=[0])
        outs[b] = res.results[0]["OUTF"]

    import threading
    threads = [threading.Thread(target=run_core, args=(b,)) for b in range(B)]
    for t in threads:
        t.start()
    for t in threads:
        t.join()

    import os
    prof_dir = os.environ.get("KERNEL_PROFILE")
    if prof_dir:
        global LAST_EXEC_NS
        runs_by_kt, nsb, in_map, perm = core_data[0]
        nc0 = _prog_cache[(runs_by_kt, nsb)]
        with jax.default_device(devices[0]):
            res = run_bass_kernel_spmd(nc0, [in_map], core_ids=[0], trace=True,
                                       tmpdir=prof_dir)
        LAST_EXEC_NS = res.exec_time_ns

    out = np.empty((B, N, 3), np.float32)
    for b in range(B):
        perm = core_data[b][3]
        out[b][perm] = outs[b].T
    return out, labels


LAST_EXEC_NS = None
